# revision 46
# baseline (speedup 1.0000x reference)
"""Ring-attention (context-parallel) kernel for 8 TRN2 NeuronCores.

Problem: x_q [1,2048,2048], x_kv [1,8192,2048], GQA attention (16 q heads,
4 kv heads, D=128) where q occupies global positions 6144..8191 of the
8192-long key sequence (causal on the last 2048 block, full attention on
the first 6144 keys), followed by an output projection.

Strategy (sequence/context parallel, the module's native layout):
  - q rows are split into 16 strips of 128; core c owns strips {c, 15-c}
    (folded pairing -> every core attends to the same total number of keys,
    perfectly balancing the causal wedge).
  - x_kv is sequence-sharded 8 x 1024 rows; each core projects its local
    K/V shard, then one AllGather per kv-head group shares the full
    K^T / V (bf16). A tiny warmup AllGather issued at kernel start absorbs
    the CC-stream init cost; V is projected first and each group's
    AllGather triggers right after its K projection so gather g=0 is in
    flight ~60us into the kernel.
  - Projection weights are replicated (bf16).
  - Each core computes all 16 heads for its 256 q rows, then the full
    output projection for those rows -> no cross-core reduction at the end.

On-device layout notes:
  S^T[k,q] = (K^T)lhsT . (Q^T)rhs with a single 1024-col bf16 matmul per
  128-key chunk (4 heads x 256 q in one moving operand), softmax via
  chunked VectorE accumulation, denominator broadcast via ones[128,128]
  matmul + reciprocal_approx_fast, normalization fused into the
  PSUM->SBUF copy, and out^T = (Wo^T)lhsT . (AO^T).
"""

import numpy as np
import ml_dtypes

import concourse.bass as bass
import concourse.mybir as mybir
import concourse.tile as tile
from concourse import bacc, bass_utils

BF16 = ml_dtypes.bfloat16
F32 = mybir.dt.float32
BF = mybir.dt.bfloat16

N_CORES = 8
H = 16          # query heads
HKV = 4         # kv heads
D = 128         # head dim
HID = H * D     # 2048
SL = 2048       # q rows (global)
SKV = 8192      # kv rows (global)
QS = 256        # q rows per core (2 strips of 128)
LKV = SKV // N_CORES   # 1024 local kv rows
HC = HID // 128        # 16 hid chunks
KC = SKV // 128        # 64 key chunks
RANK_OFF = SKV - SL    # 6144: global position of q row 0
BND = RANK_OFF // 128  # 48: first key chunk needing a causal mask
SCALE = 1.0 / float(np.sqrt(D))
W4 = 4 * QS            # 1024: 4 heads x 256 q per kv group

_CACHE = {}


def _build():
    nc = bacc.Bacc("TRN2", target_bir_lowering=False, debug=False,
                   num_devices=N_CORES)

    xqT = nc.dram_tensor("xqT", [HID, QS], BF, kind="ExternalInput")
    xkvT = nc.dram_tensor("xkvT", [HID, LKV], BF, kind="ExternalInput")
    wqT = nc.dram_tensor("wqT", [HID, HID], BF, kind="ExternalInput")
    wkT = nc.dram_tensor("wkT", [HID, HKV * D], BF, kind="ExternalInput")
    wvT = nc.dram_tensor("wvT", [HID, HKV * D], BF, kind="ExternalInput")
    woT = nc.dram_tensor("woT", [HID, HID], BF, kind="ExternalInput")
    # causal masks, replicated across the 4 heads of a kv group:
    #   maskA: key chunks 48..55 applied to strip-0 q columns
    #   maskB: key chunks 56..63 applied to strip-1 q columns
    maskA = nc.dram_tensor("maskA", [8 * 128, 512], BF, kind="ExternalInput")
    maskB = nc.dram_tensor("maskB", [8 * 128, 512], BF, kind="ExternalInput")
    # output in [q, hid] orientation (q-strip partitions) -> no transpose
    outQ = nc.dram_tensor("outQ", [QS, HID], F32, kind="ExternalOutput")

    with tile.TileContext(nc) as tc:
        _body(nc, tc, xqT, xkvT, wqT, wkT, wvT, woT, maskA, maskB, outQ)
    nc.compile()
    return nc


def _body(nc, tc, xqT, xkvT, wqT, wkT, wvT, woT, maskA, maskB, outQ):
    from contextlib import ExitStack
    ctx = ExitStack()
    with ctx:
        const = ctx.enter_context(tc.tile_pool(name="const", bufs=1))
        persist = ctx.enter_context(tc.tile_pool(name="persist", bufs=1))
        dram = ctx.enter_context(tc.tile_pool(name="dram", bufs=1, space="DRAM"))

        ones128 = const.tile([128, 128], BF)
        nc.gpsimd.memset(ones128[:], 1.0)

        rg = [list(range(N_CORES))]

        # -------- Phase 0: warmup collective (absorbs CC init/ramp) -------
        wsrc = const.tile([128, 32], BF)
        nc.gpsimd.memset(wsrc[:], 0.0)
        bncW = dram.tile([128 * 32], BF, name="bncW", uniquify=False)
        nc.gpsimd.dma_start(
            bncW[:].rearrange("(p c) -> p c", p=128), wsrc[:])
        gathW = dram.tile([N_CORES * 128 * 32], BF, addr_space="Shared",
                          name="gathW", uniquify=False)
        nc.gpsimd.collective_compute(
            "AllGather", mybir.AluOpType.bypass, replica_groups=rg,
            ins=[bncW.opt()], outs=[gathW.opt()])

        # resident inputs
        qt_sb = persist.tile([128, H, QS], BF)        # Q^T per head
        ao_sb = persist.tile([128, HKV, W4], BF)      # normalized O^T per g
        out_acc = persist.tile([128, 2, HID], F32)    # running out[q,j] sum

        # per-g AllGather bounce buffers, K and V gathered separately so the
        # first S matmuls only wait on the (earlier, smaller) K gather:
        #   bncK[g]: K^T_g [D, LKV] partition-major; bncV[g]: V_g [LKV, D]
        bncK = [dram.tile([128 * LKV], BF, name=f"bncK{g}", uniquify=False)
                for g in range(HKV)]
        bncV = [dram.tile([128 * LKV], BF, name=f"bncV{g}", uniquify=False)
                for g in range(HKV)]
        gathK = [dram.tile([N_CORES * 128 * LKV], BF, addr_space="Shared",
                           name=f"gathK{g}", uniquify=False)
                 for g in range(HKV)]
        gathV = [dram.tile([N_CORES * 128 * LKV], BF, addr_space="Shared",
                           name=f"gathV{g}", uniquify=False)
                 for g in range(HKV)]

        # ---------------- Phase A/B/C: projections + gathers --------------
        with (
            tc.tile_pool(name="kva", bufs=1) as kva,
            tc.tile_pool(name="loc", bufs=2) as loc,
            tc.tile_pool(name="qw", bufs=64) as qw,
            tc.tile_pool(name="vps", bufs=1, space="PSUM") as vps,
            tc.tile_pool(name="kps", bufs=1, space="PSUM") as kps,
            tc.tile_pool(name="qps", bufs=1, space="PSUM") as qps,
        ):
            xkv_sb = kva.tile([128, HC, LKV], BF)
            wk_sb = kva.tile([128, HC, HKV * D], BF)
            wv_sb = kva.tile([128, HC, HKV * D], BF)
            for hc in range(HC):
                nc.sync.dma_start(
                    wv_sb[:, hc, :], wvT.ap()[hc * 128:(hc + 1) * 128, :])
                nc.sync.dma_start(
                    xkv_sb[:, hc, :], xkvT.ap()[hc * 128:(hc + 1) * 128, :])
            for hc in range(HC):
                nc.sync.dma_start(
                    wk_sb[:, hc, :], wkT.ap()[hc * 128:(hc + 1) * 128, :])

            # resident loads on the sync queue after the critical Phase A
            # stream (gpsimd queue is kept free for bounce-buffer writes so
            # the AllGather triggers fire as early as possible)
            xq_sb = persist.tile([128, HC, QS], BF)   # Q^T input, hid-chunked
            nc.sync.dma_start(
                xq_sb[:], xqT.ap().rearrange("(a p) q -> p a q", p=128))
            # all Wq chunks up front so Q-projection never waits on the DMA
            # queue and the kv-slab streams find the queue empty later
            w_ts_all = []
            for g in range(HKV):
                for hc in range(HC):
                    w_t = qw.tile([128, 512], BF, tag="wq")
                    nc.sync.dma_start(
                        w_t[:],
                        wqT.ap()[hc * 128:(hc + 1) * 128,
                                 g * 512:(g + 1) * 512])
                    w_ts_all.append(w_t)
            maskA_sb = persist.tile([128, 8, 512], BF)
            nc.sync.dma_start(
                maskA_sb[:], maskA.ap().rearrange("(a p) q -> p a q", p=128))
            maskB_sb = persist.tile([128, 8, 512], BF)
            nc.sync.dma_start(
                maskB_sb[:], maskB.ap().rearrange("(a p) q -> p a q", p=128))

            # V chunks [128 keys, 512 dv] = sum_hc xkvT[hc, chunk].T @ wvT[hc]
            # 2 concurrent PSUM accumulators with hc as the outer loop, so
            # compute pipelines with the xkv DMA stream instead of waiting
            # for the full 4 MB load
            for half in range(4):
                pss = [vps.tile([128, HKV * D], F32, tag=f"v{i}",
                                name=f"psv{i}")
                       for i in range(2)]
                for hc in range(HC):
                    for i in range(2):
                        lc = half * 2 + i
                        nc.tensor.matmul(
                            pss[i][:],
                            xkv_sb[:, hc, lc * 128:(lc + 1) * 128],
                            wv_sb[:, hc, :],
                            start=(hc == 0), stop=(hc == HC - 1))
                for i in range(2):
                    lc = half * 2 + i
                    v_loc = loc.tile([128, HKV * D], BF, tag="vloc")
                    nc.scalar.activation(
                        v_loc[:], pss[i][:],
                        mybir.ActivationFunctionType.Identity)
                    # scatter the 4 per-g column blocks into the per-g bounce
                    # tiles; V region is [LKV, D] row-major
                    for g in range(HKV):
                        nc.gpsimd.dma_start(
                            bncV[g][lc * 128 * D:(lc + 1) * 128 * D]
                            .rearrange("(p d) -> p d", p=128),
                            v_loc[:, g * D:(g + 1) * D])

            # K^T_g + AllGather_g + Q-proj head block g, interleaved so the
            # g=0 gather is on the wire as early as possible
            for g in range(HKV):
                w_ts = w_ts_all[g * HC:(g + 1) * HC]

                # K^T_g [D, LKV] = sum_hc wkT[hc, g].T @ xkvT[hc]
                # (moving operand is ISA-capped at 512 elements per matmul)
                ps = kps.tile([128, LKV], F32, tag="kt")
                for hc in range(HC):
                    for nn in range(0, LKV, 512):
                        nc.tensor.matmul(
                            ps[:, nn:nn + 512],
                            wk_sb[:, hc, g * D:(g + 1) * D],
                            xkv_sb[:, hc, nn:nn + 512],
                            start=(hc == 0), stop=(hc == HC - 1))
                kt_loc = loc.tile([128, LKV], BF, tag="ktloc")
                nc.scalar.activation(
                    kt_loc[:], ps[:], mybir.ActivationFunctionType.Identity)
                nc.sync.dma_start(
                    bncK[g][:].rearrange("(p c) -> p c", p=128),
                    kt_loc[:])
                nc.gpsimd.collective_compute(
                    "AllGather", mybir.AluOpType.bypass, replica_groups=rg,
                    ins=[bncK[g].opt()], outs=[gathK[g].opt()])
                nc.gpsimd.collective_compute(
                    "AllGather", mybir.AluOpType.bypass, replica_groups=rg,
                    ins=[bncV[g].opt()], outs=[gathV[g].opt()])

                # Q projection for heads 4g..4g+3; one full PSUM bank per
                # head -- interleaved accumulation groups may not share a
                # 2KB zero region
                qp = qps.tile([128, 4, 512], F32, tag="q")
                for hc in range(HC):
                    for hh in range(4):
                        nc.tensor.matmul(
                            qp[:, hh, 0:QS],
                            w_ts[hc][:, hh * 128:(hh + 1) * 128],
                            xq_sb[:, hc, :],
                            start=(hc == 0), stop=(hc == HC - 1))
                nc.scalar.activation(
                    qt_sb[:, g * 4:(g + 1) * 4, :], qp[:, :, 0:QS],
                    mybir.ActivationFunctionType.Identity)

        # ---------------- Phase D: attention ------------------------------
        with (
            tc.tile_pool(name="kvstream", bufs=4) as kvstream,
            tc.tile_pool(name="attw", bufs=3) as attw,
            tc.tile_pool(name="wop", bufs=2) as wop,
            tc.tile_pool(name="accp", bufs=1) as accp,
            tc.tile_pool(name="stps", bufs=2, space="PSUM") as stps,
            tc.tile_pool(name="otps", bufs=1, space="PSUM") as otps,
            tc.tile_pool(name="fps", bufs=2, space="PSUM") as fps,
        ):
            for g in range(HKV):
                qt4 = qt_sb[:, g * 4:(g + 1) * 4, :]
                # prefetch this group's Wo rows on the (idle) gpsimd queue so
                # the fold at the end of the group never waits on it
                wo_g = wop.tile([128, 4, HID], BF, tag="wog")
                nc.gpsimd.dma_start(
                    wo_g[:],
                    woT.ap()[g * 512:(g + 1) * 512, :]
                    .rearrange("(a p) d -> p a d", p=128))
                ot_ps = otps.tile([128, W4], F32, tag="ot")
                # bf16 denominator accumulator, 2 chunk-halves wide (halves
                # are summed exactly by the f32 ones-matmul below)
                acc2 = accp.tile([128, 2, W4], BF, tag="acc")
                for r in range(N_CORES):
                    # stream rank r's K^T / V slabs for this head group
                    base = r * 128 * LKV
                    kt_slab = kvstream.tile([128, LKV], BF, tag="kt")
                    nc.sync.dma_start(
                        kt_slab[:],
                        gathK[g][base:base + 128 * LKV]
                        .rearrange("(p c) -> p c", p=128))
                    v_slab = kvstream.tile([128, LKV // 128, D], BF, tag="v")
                    nc.sync.dma_start(
                        v_slab[:],
                        gathV[g][base:base + 128 * LKV]
                        .rearrange("(a p d) -> p a d", p=128, d=D))
                    for l2 in range(LKV // 256):      # pairs of key chunks
                        if r == N_CORES - 1:
                            # kc >= 56: strip-0 q cols fully masked on every
                            # core -> compact half-width compute (no garbage)
                            exh = attw.tile([128, 2, 512], BF, tag="exh")
                            for eps in range(2):
                                l = l2 * 2 + eps
                                kc = r * (LKV // 128) + l
                                st = stps.tile([128, W4], F32, tag="st")
                                nc.tensor.matmul(
                                    st[:, 0:512],
                                    kt_slab[:, l * 128:(l + 1) * 128],
                                    qt4[:, :, 128:QS],
                                    start=True, stop=True)
                                nc.scalar.activation(
                                    exh[:, eps, :], st[:, 0:512],
                                    mybir.ActivationFunctionType.Exp,
                                    scale=SCALE)
                                nc.vector.tensor_mul(
                                    exh[:, eps, :], exh[:, eps, :],
                                    maskB_sb[:, l, :])
                                nc.tensor.matmul(
                                    ot_ps.rearrange(
                                        "p (h q) -> p h q", q=QS)[:, :, 128:],
                                    v_slab[:, l, :],
                                    exh[:, eps, :],
                                    start=False, stop=(kc == KC - 1))
                            # denominator: add into the strip-1 columns
                            nc.vector.tensor_add(
                                acc2.rearrange(
                                    "p e (h q) -> p e h q", q=QS)[:, :, :, 128:],
                                acc2.rearrange(
                                    "p e (h q) -> p e h q", q=QS)[:, :, :, 128:],
                                exh[:].rearrange(
                                    "p e (h q) -> p e h q", q=128))
                        else:
                            ex = attw.tile([128, 2, W4], BF, tag="ex")
                            for eps in range(2):
                                l = l2 * 2 + eps
                                kc = r * (LKV // 128) + l
                                st = stps.tile([128, W4], F32, tag="st")
                                for nn in range(2):
                                    nc.tensor.matmul(
                                        st[:, nn * 512:(nn + 1) * 512],
                                        kt_slab[:, l * 128:(l + 1) * 128],
                                        qt4[:, nn * 2:(nn + 1) * 2, :],
                                        start=True, stop=True)
                                nc.scalar.activation(
                                    ex[:, eps, :], st[:],
                                    mybir.ActivationFunctionType.Exp,
                                    scale=SCALE)
                                if r == N_CORES - 2:
                                    # kc in 48..55: mask strip-0 q columns
                                    nc.vector.tensor_mul(
                                        ex[:, eps, :].rearrange(
                                            "p (h q) -> p h q",
                                            q=QS)[:, :, 0:128],
                                        ex[:, eps, :].rearrange(
                                            "p (h q) -> p h q",
                                            q=QS)[:, :, 0:128],
                                        maskA_sb[:, l, :].rearrange(
                                            "p (h q) -> p h q", q=128))
                                for nn in range(0, W4, 512):
                                    nc.tensor.matmul(
                                        ot_ps[:, nn:nn + 512],
                                        v_slab[:, l, :],
                                        ex[:, eps, nn:nn + 512],
                                        start=(kc == 0), stop=False)
                            if r == 0 and l2 == 0:
                                nc.vector.tensor_copy(acc2[:], ex[:])
                            else:
                                nc.vector.tensor_add(acc2[:], acc2[:], ex[:])

                # denominator -> broadcast [128, W4] via ones-matmul (PSUM,
                # st tag ring), fast-approx reciprocal, then normalization
                # fused into the PSUM->SBUF copy of the attention output
                den = stps.tile([128, W4], F32, tag="st")
                for nn in range(0, W4, 512):
                    nc.tensor.matmul(den[:, nn:nn + 512], ones128[:],
                                     acc2[:, 0, nn:nn + 512],
                                     start=True, stop=False)
                    nc.tensor.matmul(den[:, nn:nn + 512], ones128[:],
                                     acc2[:, 1, nn:nn + 512],
                                     start=False, stop=True)
                rec = attw.tile([128, W4], F32, tag="rec")
                nc.vector.reciprocal_approx_fast(rec[:], den[:])
                nc.vector.tensor_mul(ao_sb[:, g, :], ot_ps[:], rec[:])

                # fold this head group into the output projection now (PE has
                # slack during the next group's attention): out[q, j] with
                # the normalized ao strip as the (reused) stationary operand
                # and Wo rows as 512-col moving operands
                for s in range(2):
                    for nn in range(HID // 512):
                        fp = fps.tile([128, 512], F32, tag="fp")
                        for hh in range(4):
                            nc.tensor.matmul(
                                fp[:],
                                ao_sb[:, g,
                                      hh * QS + s * 128:hh * QS + s * 128 + 128],
                                wo_g[:, hh, nn * 512:(nn + 1) * 512],
                                start=(hh == 0), stop=(hh == 3))
                        if g == 0:
                            nc.vector.tensor_copy(
                                out_acc[:, s, nn * 512:(nn + 1) * 512], fp[:])
                        else:
                            nc.vector.tensor_add(
                                out_acc[:, s, nn * 512:(nn + 1) * 512],
                                out_acc[:, s, nn * 512:(nn + 1) * 512], fp[:])
                        if g == HKV - 1:
                            # stream the finished output chunk out right away
                            # instead of serializing all stores at the end
                            nc.sync.dma_start(
                                outQ.ap()[s * 128:(s + 1) * 128,
                                          nn * 512:(nn + 1) * 512],
                                out_acc[:, s, nn * 512:(nn + 1) * 512])




def _get_nc():
    if "nc" not in _CACHE:
        _CACHE["nc"] = _build()
    return _CACHE["nc"]


def _make_in_maps(x_q, x_kv, Wq, Wk, Wv, Wo):
    xqT_full = np.ascontiguousarray(x_q[0].T)           # [HID, SL]
    xkvT_full = np.ascontiguousarray(x_kv[0].T)         # [HID, SKV]
    wqT = np.ascontiguousarray(Wq.T).astype(BF16)
    wkT = np.ascontiguousarray(Wk.T).astype(BF16)
    wvT = np.ascontiguousarray(Wv.T).astype(BF16)
    woT = np.ascontiguousarray(Wo.T).astype(BF16)

    kk = np.arange(128)
    in_maps = []
    for c in range(N_CORES):
        s0, s1 = c, 15 - c
        xqT = np.concatenate(
            [xqT_full[:, s0 * 128:(s0 + 1) * 128],
             xqT_full[:, s1 * 128:(s1 + 1) * 128]], axis=1).astype(BF16)
        xkvT = np.ascontiguousarray(
            xkvT_full[:, c * LKV:(c + 1) * LKV]).astype(BF16)
        # maskA: key chunks 48..55 vs strip-0 q rows, replicated x4 heads
        mA = np.zeros((8, 128, 128), dtype=np.float32)
        q0 = RANK_OFF + s0 * 128 + np.arange(128)
        q1 = RANK_OFF + s1 * 128 + np.arange(128)
        for j in range(8):
            key_g = (BND + j) * 128 + kk
            mA[j] = key_g[:, None] <= q0[None, :]
        # maskB: key chunks 56..63 vs strip-1 q rows
        mB = np.zeros((8, 128, 128), dtype=np.float32)
        for j in range(8):
            key_g = (56 + j) * 128 + kk
            mB[j] = key_g[:, None] <= q1[None, :]
        mA4 = np.tile(mA, (1, 1, 4)).reshape(8 * 128, 512)
        mB4 = np.tile(mB, (1, 1, 4)).reshape(8 * 128, 512)
        in_maps.append({
            "xqT": xqT, "xkvT": xkvT, "wqT": wqT, "wkT": wkT,
            "wvT": wvT, "woT": woT,
            "maskA": mA4.astype(BF16), "maskB": mB4.astype(BF16),
        })
    return in_maps


def _unshard(results):
    out = np.empty((1, SL, HID), dtype=np.float32)
    for c in range(N_CORES):
        outQ = results[c]["outQ"]                       # [QS, HID]
        s0, s1 = c, 15 - c
        out[0, s0 * 128:(s0 + 1) * 128, :] = outQ[0:128]
        out[0, s1 * 128:(s1 + 1) * 128, :] = outQ[128:256]
    return out


def kernel(x_q, x_kv, Wq, Wk, Wv, Wo, _trace=False, _result_box=None):
    nc = _get_nc()
    in_maps = _make_in_maps(x_q, x_kv, Wq, Wk, Wv, Wo)
    res = bass_utils.run_bass_kernel_spmd(
        nc, in_maps, core_ids=list(range(N_CORES)), trace=_trace)
    if _result_box is not None:
        _result_box.append(res)
    return _unshard(res.results)


# revision 47
# speedup vs baseline: 1.1139x; 1.1139x over previous
"""Ring-attention (context-parallel) kernel for 8 TRN2 NeuronCores.

Problem: x_q [1,2048,2048], x_kv [1,8192,2048], GQA attention (16 q heads,
4 kv heads, D=128) where q occupies global positions 6144..8191 of the
8192-long key sequence (causal on the last 2048 block, full attention on
the first 6144 keys), followed by an output projection.

Strategy (sequence/context parallel, the module's native layout):
  - q rows are split into 16 strips of 128; core c owns strips {c, 15-c}
    (folded pairing -> every core attends to the same total number of keys,
    perfectly balancing the causal wedge).
  - x_kv is sequence-sharded 8 x 1024 rows; each core projects its local
    K/V shard, then one AllGather per kv-head group shares the full
    K^T / V (bf16). A tiny warmup AllGather issued at kernel start absorbs
    the CC-stream init cost; V is projected first and each group's
    AllGather triggers right after its K projection so gather g=0 is in
    flight ~60us into the kernel.
  - Projection weights are replicated (bf16).
  - Each core computes all 16 heads for its 256 q rows, then the full
    output projection for those rows -> no cross-core reduction at the end.

On-device layout notes:
  S^T[k,q] = (K^T)lhsT . (Q^T)rhs with a single 1024-col bf16 matmul per
  128-key chunk (4 heads x 256 q in one moving operand), softmax via
  chunked VectorE accumulation, denominator broadcast via ones[128,128]
  matmul + reciprocal_approx_fast, normalization fused into the
  PSUM->SBUF copy, and out^T = (Wo^T)lhsT . (AO^T).
"""

import numpy as np
import ml_dtypes

import concourse.bass as bass
import concourse.mybir as mybir
import concourse.tile as tile
from concourse import bacc, bass_utils

BF16 = ml_dtypes.bfloat16
F32 = mybir.dt.float32
BF = mybir.dt.bfloat16

N_CORES = 8
H = 16          # query heads
HKV = 4         # kv heads
D = 128         # head dim
HID = H * D     # 2048
SL = 2048       # q rows (global)
SKV = 8192      # kv rows (global)
QS = 256        # q rows per core (2 strips of 128)
LKV = SKV // N_CORES   # 1024 local kv rows
HC = HID // 128        # 16 hid chunks
KC = SKV // 128        # 64 key chunks
RANK_OFF = SKV - SL    # 6144: global position of q row 0
BND = RANK_OFF // 128  # 48: first key chunk needing a causal mask
SCALE = 1.0 / float(np.sqrt(D))
W4 = 4 * QS            # 1024: 4 heads x 256 q per kv group

_CACHE = {}


def _build():
    nc = bacc.Bacc("TRN2", target_bir_lowering=False, debug=False,
                   num_devices=N_CORES)

    xqT = nc.dram_tensor("xqT", [HID, QS], BF, kind="ExternalInput")
    xkvT = nc.dram_tensor("xkvT", [HID, LKV], BF, kind="ExternalInput")
    wqT = nc.dram_tensor("wqT", [HID, HID], BF, kind="ExternalInput")
    wkT = nc.dram_tensor("wkT", [HID, HKV * D], BF, kind="ExternalInput")
    wvT = nc.dram_tensor("wvT", [HID, HKV * D], BF, kind="ExternalInput")
    woT = nc.dram_tensor("woT", [HID, HID], BF, kind="ExternalInput")
    # causal masks, replicated across the 4 heads of a kv group:
    #   maskA: key chunks 48..55 applied to strip-0 q columns
    #   maskB: key chunks 56..63 applied to strip-1 q columns
    maskA = nc.dram_tensor("maskA", [8 * 128, 512], BF, kind="ExternalInput")
    maskB = nc.dram_tensor("maskB", [8 * 128, 512], BF, kind="ExternalInput")
    # output in [q, hid] orientation (q-strip partitions) -> no transpose
    outQ = nc.dram_tensor("outQ", [QS, HID], F32, kind="ExternalOutput")

    with tile.TileContext(nc) as tc:
        _body(nc, tc, xqT, xkvT, wqT, wkT, wvT, woT, maskA, maskB, outQ)
    nc.compile()
    return nc


def _body(nc, tc, xqT, xkvT, wqT, wkT, wvT, woT, maskA, maskB, outQ):
    from contextlib import ExitStack
    ctx = ExitStack()
    with ctx:
        const = ctx.enter_context(tc.tile_pool(name="const", bufs=1))
        persist = ctx.enter_context(tc.tile_pool(name="persist", bufs=1))
        dram = ctx.enter_context(tc.tile_pool(name="dram", bufs=1, space="DRAM"))

        ones128 = const.tile([128, 128], BF)
        nc.gpsimd.memset(ones128[:], 1.0)

        rg = [list(range(N_CORES))]

        # -------- Phase 0: warmup collective (absorbs CC init/ramp) -------
        wsrc = const.tile([128, 32], BF)
        nc.gpsimd.memset(wsrc[:], 0.0)
        bncW = dram.tile([128 * 32], BF, name="bncW", uniquify=False)
        nc.gpsimd.dma_start(
            bncW[:].rearrange("(p c) -> p c", p=128), wsrc[:])
        gathW = dram.tile([N_CORES * 128 * 32], BF, addr_space="Shared",
                          name="gathW", uniquify=False)
        nc.gpsimd.collective_compute(
            "AllGather", mybir.AluOpType.bypass, replica_groups=rg,
            ins=[bncW.opt()], outs=[gathW.opt()])

        # resident inputs
        qt_sb = persist.tile([128, H, QS], BF)        # Q^T per head
        ao_sb = persist.tile([128, HKV, W4], BF)      # normalized O^T per g
        out_acc = persist.tile([128, 2, HID], F32)    # running out[q,j] sum

        # per-g AllGather bounce (1D): bytes [0 : 128*LKV) = K^T_g [D, LKV],
        # bytes [128*LKV : 256*LKV) = V_g [LKV, D] row-major
        bnc = [dram.tile([256 * LKV], BF, name=f"bnc{g}", uniquify=False)
               for g in range(HKV)]
        gath = [dram.tile([N_CORES * 256 * LKV], BF, addr_space="Shared",
                          name=f"gath{g}", uniquify=False)
                for g in range(HKV)]

        # ---------------- Phase A/B/C: projections + gathers --------------
        with (
            tc.tile_pool(name="kva", bufs=1) as kva,
            tc.tile_pool(name="loc", bufs=2) as loc,
            tc.tile_pool(name="qw", bufs=64) as qw,
            tc.tile_pool(name="vps", bufs=1, space="PSUM") as vps,
            tc.tile_pool(name="kps", bufs=1, space="PSUM") as kps,
            tc.tile_pool(name="qps", bufs=1, space="PSUM") as qps,
        ):
            xkv_sb = kva.tile([128, HC, LKV], BF)
            wk_sb = kva.tile([128, HC, HKV * D], BF)
            wv_sb = kva.tile([128, HC, HKV * D], BF)
            for hc in range(HC):
                nc.sync.dma_start(
                    wv_sb[:, hc, :], wvT.ap()[hc * 128:(hc + 1) * 128, :])
                nc.sync.dma_start(
                    xkv_sb[:, hc, :], xkvT.ap()[hc * 128:(hc + 1) * 128, :])
            for hc in range(HC):
                nc.sync.dma_start(
                    wk_sb[:, hc, :], wkT.ap()[hc * 128:(hc + 1) * 128, :])

            # resident loads on the sync queue after the critical Phase A
            # stream (gpsimd queue is kept free for bounce-buffer writes so
            # the AllGather triggers fire as early as possible)
            xq_sb = persist.tile([128, HC, QS], BF)   # Q^T input, hid-chunked
            nc.sync.dma_start(
                xq_sb[:], xqT.ap().rearrange("(a p) q -> p a q", p=128))
            # all Wq chunks up front so Q-projection never waits on the DMA
            # queue and the kv-slab streams find the queue empty later
            w_ts_all = []
            for g in range(HKV):
                for hc in range(HC):
                    w_t = qw.tile([128, 512], BF, tag="wq")
                    nc.sync.dma_start(
                        w_t[:],
                        wqT.ap()[hc * 128:(hc + 1) * 128,
                                 g * 512:(g + 1) * 512])
                    w_ts_all.append(w_t)
            maskA_sb = persist.tile([128, 8, 512], BF)
            nc.sync.dma_start(
                maskA_sb[:], maskA.ap().rearrange("(a p) q -> p a q", p=128))
            maskB_sb = persist.tile([128, 8, 512], BF)
            nc.sync.dma_start(
                maskB_sb[:], maskB.ap().rearrange("(a p) q -> p a q", p=128))

            # V chunks [128 keys, 512 dv] = sum_hc xkvT[hc, chunk].T @ wvT[hc]
            # 2 concurrent PSUM accumulators with hc as the outer loop, so
            # compute pipelines with the xkv DMA stream instead of waiting
            # for the full 4 MB load
            for half in range(4):
                pss = [vps.tile([128, HKV * D], F32, tag=f"v{i}",
                                name=f"psv{i}")
                       for i in range(2)]
                for hc in range(HC):
                    for i in range(2):
                        lc = half * 2 + i
                        nc.tensor.matmul(
                            pss[i][:],
                            xkv_sb[:, hc, lc * 128:(lc + 1) * 128],
                            wv_sb[:, hc, :],
                            start=(hc == 0), stop=(hc == HC - 1))
                for i in range(2):
                    lc = half * 2 + i
                    v_loc = loc.tile([128, HKV * D], BF, tag="vloc")
                    nc.scalar.activation(
                        v_loc[:], pss[i][:],
                        mybir.ActivationFunctionType.Identity)
                    # scatter the 4 per-g column blocks into the per-g bounce
                    # tiles; V region is [LKV, D] row-major
                    for g in range(HKV):
                        nc.gpsimd.dma_start(
                            bnc[g][128 * LKV + lc * 128 * D:
                                   128 * LKV + (lc + 1) * 128 * D]
                            .rearrange("(p d) -> p d", p=128),
                            v_loc[:, g * D:(g + 1) * D])

            # K^T_g + AllGather_g + Q-proj head block g, interleaved so the
            # g=0 gather is on the wire as early as possible
            for g in range(HKV):
                w_ts = w_ts_all[g * HC:(g + 1) * HC]

                # K^T_g [D, LKV] = sum_hc wkT[hc, g].T @ xkvT[hc]
                # (moving operand is ISA-capped at 512 elements per matmul)
                ps = kps.tile([128, LKV], F32, tag="kt")
                for hc in range(HC):
                    for nn in range(0, LKV, 512):
                        nc.tensor.matmul(
                            ps[:, nn:nn + 512],
                            wk_sb[:, hc, g * D:(g + 1) * D],
                            xkv_sb[:, hc, nn:nn + 512],
                            start=(hc == 0), stop=(hc == HC - 1))
                kt_loc = loc.tile([128, LKV], BF, tag="ktloc")
                nc.scalar.activation(
                    kt_loc[:], ps[:], mybir.ActivationFunctionType.Identity)
                nc.gpsimd.dma_start(
                    bnc[g][0:128 * LKV].rearrange("(p c) -> p c", p=128),
                    kt_loc[:])
                nc.gpsimd.collective_compute(
                    "AllGather", mybir.AluOpType.bypass, replica_groups=rg,
                    ins=[bnc[g].opt()], outs=[gath[g].opt()])

                # Q projection for heads 4g..4g+3; one full PSUM bank per
                # head -- interleaved accumulation groups may not share a
                # 2KB zero region
                qp = qps.tile([128, 4, 512], F32, tag="q")
                for hc in range(HC):
                    for hh in range(4):
                        nc.tensor.matmul(
                            qp[:, hh, 0:QS],
                            w_ts[hc][:, hh * 128:(hh + 1) * 128],
                            xq_sb[:, hc, :],
                            start=(hc == 0), stop=(hc == HC - 1))
                nc.scalar.activation(
                    qt_sb[:, g * 4:(g + 1) * 4, :], qp[:, :, 0:QS],
                    mybir.ActivationFunctionType.Identity)

        # ---------------- Phase D: attention ------------------------------
        with (
            tc.tile_pool(name="kvstream", bufs=4) as kvstream,
            tc.tile_pool(name="attw", bufs=3) as attw,
            tc.tile_pool(name="wop", bufs=2) as wop,
            tc.tile_pool(name="accp", bufs=1) as accp,
            tc.tile_pool(name="stps", bufs=2, space="PSUM") as stps,
            tc.tile_pool(name="otps", bufs=1, space="PSUM") as otps,
            tc.tile_pool(name="fps", bufs=2, space="PSUM") as fps,
        ):
            for g in range(HKV):
                qt4 = qt_sb[:, g * 4:(g + 1) * 4, :]
                # prefetch this group's Wo rows on the (idle) gpsimd queue so
                # the fold at the end of the group never waits on it
                wo_g = wop.tile([128, 4, HID], BF, tag="wog")
                nc.gpsimd.dma_start(
                    wo_g[:],
                    woT.ap()[g * 512:(g + 1) * 512, :]
                    .rearrange("(a p) d -> p a d", p=128))
                ot_ps = otps.tile([128, W4], F32, tag="ot")
                # bf16 denominator accumulator, 2 chunk-halves wide (halves
                # are summed exactly by the f32 ones-matmul below)
                acc2 = accp.tile([128, 2, W4], BF, tag="acc")
                for r in range(N_CORES):
                    # stream rank r's K^T / V slabs for this head group
                    base = r * 256 * LKV
                    kt_slab = kvstream.tile([128, LKV], BF, tag="kt")
                    nc.sync.dma_start(
                        kt_slab[:],
                        gath[g][base:base + 128 * LKV]
                        .rearrange("(p c) -> p c", p=128))
                    v_slab = kvstream.tile([128, LKV // 128, D], BF, tag="v")
                    nc.sync.dma_start(
                        v_slab[:],
                        gath[g][base + 128 * LKV:base + 256 * LKV]
                        .rearrange("(a p d) -> p a d", p=128, d=D))
                    for l2 in range(LKV // 256):      # pairs of key chunks
                        if r == N_CORES - 1:
                            # kc >= 56: strip-0 q cols fully masked on every
                            # core -> compact half-width compute (no garbage)
                            exh = attw.tile([128, 2, 512], BF, tag="exh")
                            for eps in range(2):
                                l = l2 * 2 + eps
                                kc = r * (LKV // 128) + l
                                st = stps.tile([128, W4], F32, tag="st")
                                nc.tensor.matmul(
                                    st[:, 0:512],
                                    kt_slab[:, l * 128:(l + 1) * 128],
                                    qt4[:, :, 128:QS],
                                    start=True, stop=True)
                                nc.scalar.activation(
                                    exh[:, eps, :], st[:, 0:512],
                                    mybir.ActivationFunctionType.Exp,
                                    scale=SCALE)
                                nc.vector.tensor_mul(
                                    exh[:, eps, :], exh[:, eps, :],
                                    maskB_sb[:, l, :])
                                nc.tensor.matmul(
                                    ot_ps.rearrange(
                                        "p (h q) -> p h q", q=QS)[:, :, 128:],
                                    v_slab[:, l, :],
                                    exh[:, eps, :],
                                    start=False, stop=(kc == KC - 1))
                            # denominator: add into the strip-1 columns
                            nc.vector.tensor_add(
                                acc2.rearrange(
                                    "p e (h q) -> p e h q", q=QS)[:, :, :, 128:],
                                acc2.rearrange(
                                    "p e (h q) -> p e h q", q=QS)[:, :, :, 128:],
                                exh[:].rearrange(
                                    "p e (h q) -> p e h q", q=128))
                        else:
                            ex = attw.tile([128, 2, W4], BF, tag="ex")
                            for eps in range(2):
                                l = l2 * 2 + eps
                                kc = r * (LKV // 128) + l
                                st = stps.tile([128, W4], F32, tag="st")
                                for nn in range(2):
                                    nc.tensor.matmul(
                                        st[:, nn * 512:(nn + 1) * 512],
                                        kt_slab[:, l * 128:(l + 1) * 128],
                                        qt4[:, nn * 2:(nn + 1) * 2, :],
                                        start=True, stop=True)
                                nc.scalar.activation(
                                    ex[:, eps, :], st[:],
                                    mybir.ActivationFunctionType.Exp,
                                    scale=SCALE)
                                if r == N_CORES - 2:
                                    # kc in 48..55: mask strip-0 q columns
                                    nc.vector.tensor_mul(
                                        ex[:, eps, :].rearrange(
                                            "p (h q) -> p h q",
                                            q=QS)[:, :, 0:128],
                                        ex[:, eps, :].rearrange(
                                            "p (h q) -> p h q",
                                            q=QS)[:, :, 0:128],
                                        maskA_sb[:, l, :].rearrange(
                                            "p (h q) -> p h q", q=128))
                                for nn in range(0, W4, 512):
                                    nc.tensor.matmul(
                                        ot_ps[:, nn:nn + 512],
                                        v_slab[:, l, :],
                                        ex[:, eps, nn:nn + 512],
                                        start=(kc == 0), stop=False)
                            if r == 0 and l2 == 0:
                                nc.vector.tensor_copy(acc2[:], ex[:])
                            else:
                                nc.vector.tensor_add(acc2[:], acc2[:], ex[:])

                # denominator -> broadcast [128, W4] via ones-matmul (PSUM,
                # st tag ring), fast-approx reciprocal, then normalization
                # fused into the PSUM->SBUF copy of the attention output
                den = stps.tile([128, W4], F32, tag="st")
                for nn in range(0, W4, 512):
                    nc.tensor.matmul(den[:, nn:nn + 512], ones128[:],
                                     acc2[:, 0, nn:nn + 512],
                                     start=True, stop=False)
                    nc.tensor.matmul(den[:, nn:nn + 512], ones128[:],
                                     acc2[:, 1, nn:nn + 512],
                                     start=False, stop=True)
                rec = attw.tile([128, W4], F32, tag="rec")
                nc.vector.reciprocal_approx_fast(rec[:], den[:])
                nc.vector.tensor_mul(ao_sb[:, g, :], ot_ps[:], rec[:])

                # fold this head group into the output projection now (PE has
                # slack during the next group's attention): out[q, j] with
                # the normalized ao strip as the (reused) stationary operand
                # and Wo rows as 512-col moving operands
                for s in range(2):
                    for nn in range(HID // 512):
                        fp = fps.tile([128, 512], F32, tag="fp")
                        for hh in range(4):
                            nc.tensor.matmul(
                                fp[:],
                                ao_sb[:, g,
                                      hh * QS + s * 128:hh * QS + s * 128 + 128],
                                wo_g[:, hh, nn * 512:(nn + 1) * 512],
                                start=(hh == 0), stop=(hh == 3))
                        if g == 0:
                            nc.vector.tensor_copy(
                                out_acc[:, s, nn * 512:(nn + 1) * 512], fp[:])
                        else:
                            nc.vector.tensor_add(
                                out_acc[:, s, nn * 512:(nn + 1) * 512],
                                out_acc[:, s, nn * 512:(nn + 1) * 512], fp[:])
                        if g == HKV - 1:
                            # stream the finished output chunk out right away
                            # instead of serializing all stores at the end
                            nc.sync.dma_start(
                                outQ.ap()[s * 128:(s + 1) * 128,
                                          nn * 512:(nn + 1) * 512],
                                out_acc[:, s, nn * 512:(nn + 1) * 512])




def _get_nc():
    if "nc" not in _CACHE:
        _CACHE["nc"] = _build()
    return _CACHE["nc"]


def _make_in_maps(x_q, x_kv, Wq, Wk, Wv, Wo):
    xqT_full = np.ascontiguousarray(x_q[0].T)           # [HID, SL]
    xkvT_full = np.ascontiguousarray(x_kv[0].T)         # [HID, SKV]
    wqT = np.ascontiguousarray(Wq.T).astype(BF16)
    wkT = np.ascontiguousarray(Wk.T).astype(BF16)
    wvT = np.ascontiguousarray(Wv.T).astype(BF16)
    woT = np.ascontiguousarray(Wo.T).astype(BF16)

    kk = np.arange(128)
    in_maps = []
    for c in range(N_CORES):
        s0, s1 = c, 15 - c
        xqT = np.concatenate(
            [xqT_full[:, s0 * 128:(s0 + 1) * 128],
             xqT_full[:, s1 * 128:(s1 + 1) * 128]], axis=1).astype(BF16)
        xkvT = np.ascontiguousarray(
            xkvT_full[:, c * LKV:(c + 1) * LKV]).astype(BF16)
        # maskA: key chunks 48..55 vs strip-0 q rows, replicated x4 heads
        mA = np.zeros((8, 128, 128), dtype=np.float32)
        q0 = RANK_OFF + s0 * 128 + np.arange(128)
        q1 = RANK_OFF + s1 * 128 + np.arange(128)
        for j in range(8):
            key_g = (BND + j) * 128 + kk
            mA[j] = key_g[:, None] <= q0[None, :]
        # maskB: key chunks 56..63 vs strip-1 q rows
        mB = np.zeros((8, 128, 128), dtype=np.float32)
        for j in range(8):
            key_g = (56 + j) * 128 + kk
            mB[j] = key_g[:, None] <= q1[None, :]
        mA4 = np.tile(mA, (1, 1, 4)).reshape(8 * 128, 512)
        mB4 = np.tile(mB, (1, 1, 4)).reshape(8 * 128, 512)
        in_maps.append({
            "xqT": xqT, "xkvT": xkvT, "wqT": wqT, "wkT": wkT,
            "wvT": wvT, "woT": woT,
            "maskA": mA4.astype(BF16), "maskB": mB4.astype(BF16),
        })
    return in_maps


def _unshard(results):
    out = np.empty((1, SL, HID), dtype=np.float32)
    for c in range(N_CORES):
        outQ = results[c]["outQ"]                       # [QS, HID]
        s0, s1 = c, 15 - c
        out[0, s0 * 128:(s0 + 1) * 128, :] = outQ[0:128]
        out[0, s1 * 128:(s1 + 1) * 128, :] = outQ[128:256]
    return out


def kernel(x_q, x_kv, Wq, Wk, Wv, Wo, _trace=False, _result_box=None):
    nc = _get_nc()
    in_maps = _make_in_maps(x_q, x_kv, Wq, Wk, Wv, Wo)
    res = bass_utils.run_bass_kernel_spmd(
        nc, in_maps, core_ids=list(range(N_CORES)), trace=_trace)
    if _result_box is not None:
        _result_box.append(res)
    return _unshard(res.results)
